# revision 23
# baseline (speedup 1.0000x reference)
"""BertTinyFlatten on 8 Trainium2 NeuronCores — data-parallel over batch.

Host prep folds the (tiny, ~0.01% of FLOPs) embedding gather + layernorm
into input packing: each core receives x0t = LN(word_emb[x]+pos+tok)^T
pre-transposed into feature-major s-chunk panels, plus the five matmul
operands host-prelaid in PE-friendly layouts.  The device program is a
pure matmul pipeline (99.99% of the reference FLOPs):

  y1    = x0 @ init_d.T            (token-major out)      [bf16 matmuls]
  y1sq  = (mix(y1, init_M) + b1)^2   -> feature-major     [ACT Square fused]
  y2    = y1sq @ inter0_d.T          -> token-major
  y2sq  = (mix(y2, inter0_M) + b2)^2 -> feature-major
  yt    = final_d @ y2sq + b3        -> feature-major; host transposes back

All matmul operands are bf16 (PSUM accumulates fp32): the moving operand
streams 1 col/cycle and the 128-col LDWEIGHTS rides the background weight
buffer, giving the 216 ns warm pitch at N=512.  PSUM->SBUF drains
alternate between the Scalar and Vector engines so neither queue backs up
ahead of the next stage's gated matmuls.  All DMA triggers ride the Sync
engine stream: triggers issue in program order and the HWDGE queue is
FIFO, which is what keeps the bulk panels (m1/m2 ring-prefetched 3 deep,
w2/w3 from mix1/dense2 tail hooks) behind the critical dense1 operands —
a dep-free trigger on any other engine gets hoisted to t=0 by the Tile
scheduler and starves the critical path.  A short burst of junk matmuls
ramps the PE clock (DVFS) while the first operands land, so dense1 runs
at the warm pitch from its first matmul.  Mid-kernel the PE has zero
gaps; the remaining overhead is first-operand DMA latency (~6.5us) and
the fixed runtime teardown (~11us).
"""
import os
import sys

import numpy as np
import ml_dtypes

for _p in ("/opt/trn_rl_repo", "/opt/pypackages"):
    if _p not in sys.path and os.path.isdir(_p):
        sys.path.append(_p)

from contextlib import ExitStack

import concourse.bass as bass
import concourse.tile as tile
from concourse import bacc, masks, mybir
from concourse.bass_utils import run_bass_kernel_spmd

f32 = mybir.dt.float32
bf16 = mybir.dt.bfloat16
AF = mybir.ActivationFunctionType
ALU = mybir.AluOpType

B, S, HID, NH, INTER, VOCAB = 8, 1024, 512, 8, 2048, 30522
DH = INTER // NH            # 256 features per head
EPS = 1e-12
N_CORES = 8

KH = HID // 128             # 4   k-tiles for dense1
KI = INTER // 128           # 16  k-tiles for dense2/3
SC = S // 128               # 8   token chunks
NC1 = INTER // 512          # 4   n-chunks (512) for dense1/2
HT = HID // 128             # 4   hid tiles for dense3

STAGES = ("A", "B", "C", "D", "E", "full")


def _build_program(stage="full"):
    upto = STAGES.index(stage)
    nc = bacc.Bacc("TRN2", target_bir_lowering=False, debug=False,
                   num_devices=N_CORES)

    # prelaid panels (host does the tiling): see _prep_maps for layouts
    x0s = nc.dram_tensor("x0s", [128, SC * KH * 128], bf16,
                         kind="ExternalInput").ap()
    # biases catted: [0:16]=b1, [16:32]=b2, [32:36]=b3
    bcat = nc.dram_tensor("bcat", [128, 2 * KI + HT], f32,
                          kind="ExternalInput").ap()
    w1 = nc.dram_tensor("w1", [128, NC1 * KH * 512], bf16, kind="ExternalInput").ap()
    m1 = nc.dram_tensor("m1", [NH * 128, SC * S], bf16, kind="ExternalInput").ap()
    w2 = nc.dram_tensor("w2", [NC1 * 128, KI * 512], bf16, kind="ExternalInput").ap()
    m2 = nc.dram_tensor("m2", [NH * 128, SC * S], bf16, kind="ExternalInput").ap()
    w3 = nc.dram_tensor("w3", [128, KI * 512], bf16, kind="ExternalInput").ap()
    yt_out = nc.dram_tensor("yt", [HID, S], bf16, kind="ExternalOutput").ap()

    with tile.TileContext(nc) as tc, ExitStack() as ctx:
        pool = ctx.enter_context(tc.tile_pool(name="sbuf", bufs=1))
        psum = ctx.enter_context(tc.tile_pool(name="psum", bufs=1, space="PSUM"))

        # ---- critical-path DMAs first (Sync HWDGE queue, FIFO), ordered
        # by first consumption.  dense1 runs n-outer/s-inner, so pass 0
        # needs x0s chunks progressively (128KB each) and each later w1
        # n-panel has a full 6.9us pass of runway.  All triggers ride the
        # Sync engine stream: it issues in program order and the queue is
        # FIFO, which keeps the m panels behind the dense1 operands (the
        # Tile scheduler hoists dep-free triggers on any other engine to
        # t=0, starving the critical path).  Biases are first read ~40us in.
        x0sb = [pool.tile([128, KH * 128], bf16, tag="x0s", bufs=SC,
                          name=f"x0s{s}") for s in range(SC)]
        w1sb = pool.tile([128, NC1 * KH * 512], bf16)
        nc.sync.dma_start(x0sb[0][:], x0s[:, 0:512])
        nc.sync.dma_start(w1sb[:, 0:2048], w1[:, 0:2048])
        for s in range(1, SC):
            nc.sync.dma_start(x0sb[s][:], x0s[:, s * 512:(s + 1) * 512])
        for n in range(1, NC1):
            nc.sync.dma_start(w1sb[:, n * 2048:(n + 1) * 2048],
                              w1[:, n * 2048:(n + 1) * 2048])

        # m panels ride the same FIFO queue behind the dense1 operands; the
        # bufs=3 ring WAR-gates the h+3 refill triggers during the mixes.
        def m_panel(m_ap, h, nm):
            t = pool.tile([128, SC * S], bf16, tag="m", bufs=3, name=f"{nm}p{h}")
            nc.sync.dma_start(t[:], m_ap[h * 128:(h + 1) * 128, :])
            return t

        m1pan = [m_panel(m1, 0, "m1")]
        t_bc = pool.tile([128, 2 * KI + HT], f32)
        nc.sync.dma_start(t_bc[:], bcat[:])
        m1pan += [m_panel(m1, 1, "m1"), m_panel(m1, 2, "m1")]

        # ---- constants + PE/ACT warmup ----------------------------------
        ident = pool.tile([128, 128], bf16)
        masks.make_identity(nc, ident[:])
        zerocol = pool.tile([128, 1], f32)
        nc.vector.memset(zerocol[:], 0.0)
        junk = pool.tile([128, 512], bf16)
        nc.vector.memset(junk[:], 0.0)
        # column bases into t_bc: b1, b2, b3
        B1, B2, B3 = 0, KI, 2 * KI

        def bc(base, i):
            return t_bc[:, base + i:base + i + 1]

        # dummy Square so walrus loads the square/identity table set once,
        # off the critical path
        actwarm = pool.tile([128, 1], f32)
        nc.scalar.activation(actwarm[:], zerocol[:], AF.Square, bias=zerocol[:])

        # junk matmuls ramp the PE clock while x0s0/w1n0 land (~4.5us at
        # the slow early DMA rate); sized to cover the arrival jitter so the
        # PE never idles before dense1 — an idle gap there lets the DVFS
        # clock sag and costs ~2.5us of half-speed dense1 matmuls
        warmps = psum.tile([128, 512], f32, tag="mm", bufs=8, name="warm")
        for i in range(11):
            nc.tensor.matmul(warmps[:], ident[:], junk[:], start=True, stop=True)

        # feature-major activations live in the 16-slot "featmaj" ring:
        # y1sq (16) -> y2sq (16), WAR-serialized by Tile.
        def mix(yin, m_ap, panels, bias_base, out_name, tail_hooks=None):
            # per-head seq mix + bias + square; token-major in, feature-major out
            ysq = []
            for h in range(NH):
                pan = panels[h]
                groups = [[None] * 2 for _ in range(2)]
                for tc_i in range(2):
                    for dp in range(2):
                        groups[tc_i][dp] = psum.tile(
                            [128, 512], f32, tag="mm", bufs=8,
                            name=f"{out_name}p{h}_{tc_i}_{dp}")
                for s in range(SC):
                    for dp in range(2):
                        lhsT = yin[s][:, h * DH + dp * 128: h * DH + (dp + 1) * 128]
                        for tc_i in range(2):
                            nc.tensor.matmul(
                                groups[tc_i][dp][:], lhsT,
                                pan[:, s * S + tc_i * 512: s * S + (tc_i + 1) * 512],
                                start=(s == 0), stop=(s == SC - 1))
                if h + 3 < NH:
                    panels.append(m_panel(m_ap, h + 3, out_name))
                if tail_hooks and h in tail_hooks:
                    for fn in tail_hooks[h]:
                        fn()
                for dp in range(2):
                    i = h * 2 + dp
                    yo = pool.tile([128, S], bf16, tag="featmaj", bufs=16,
                                   name=f"{out_name}{i}")
                    for tc_i in range(2):
                        nc.scalar.activation(yo[:, tc_i * 512:(tc_i + 1) * 512],
                                             groups[tc_i][dp][:], AF.Square,
                                             bias=bc(bias_base, i))
                    ysq.append(yo)
            return ysq

        def w2_panel(n):
            # all bulk DMAs ride the Sync engine stream: the engine issues
            # triggers in program order and the HWDGE queue is FIFO, which is
            # the only thing keeping the panel stream from starving the
            # critical dense1 operands (the Tile scheduler hoists dep-free
            # triggers on any other engine to t=0)
            t = pool.tile([128, KI * 512], bf16, tag="w2", bufs=2, name=f"w2q{n}")
            nc.sync.dma_start(t[:], w2[n * 128:(n + 1) * 128, :])
            return t

        def dense1():
            # token-major out: y1[s] = x0[s-chunk] @ w1; n-outer so each w1
            # n-panel DMA has a full pass of matmul runway
            yt = []
            for s in range(SC):
                yt.append(pool.tile([128, INTER], bf16, tag="tokmaj", bufs=SC,
                                    name=f"y1_{s}"))
            for n in range(NC1):
                for s in range(SC):
                    ps = psum.tile([128, 512], f32, tag="mm", bufs=8,
                                   name=f"y1p{n}_{s}")
                    for k in range(KH):
                        nc.tensor.matmul(
                            ps[:], x0sb[s][:, k * 128:(k + 1) * 128],
                            w1sb[:, n * 2048 + k * 512: n * 2048 + (k + 1) * 512],
                            start=(k == 0), stop=(k == KH - 1))
                    if s % 2:
                        nc.scalar.copy(yt[s][:, n * 512:(n + 1) * 512], ps[:])
                    else:
                        nc.vector.tensor_copy(yt[s][:, n * 512:(n + 1) * 512],
                                              ps[:])
            return yt

        def dense2(xin, w2pans, tail_hooks=None):
            yt = []
            for s in range(SC):
                yt.append(pool.tile([128, INTER], bf16, tag="tokmaj", bufs=SC,
                                    name=f"y2_{s}"))
            for n in range(NC1):
                wq = w2pans[n]
                for s in range(SC):
                    ps = psum.tile([128, 512], f32, tag="mm", bufs=8,
                                   name=f"y2p{n}_{s}")
                    for k in range(KI):
                        nc.tensor.matmul(
                            ps[:], xin[k][:, s * 128:(s + 1) * 128],
                            wq[:, k * 512:(k + 1) * 512],
                            start=(k == 0), stop=(k == KI - 1))
                    if s % 2:
                        nc.scalar.copy(yt[s][:, n * 512:(n + 1) * 512], ps[:])
                    else:
                        nc.vector.tensor_copy(yt[s][:, n * 512:(n + 1) * 512],
                                              ps[:])
                if n + 2 < NC1:
                    w2pans.append(w2_panel(n + 2))
                if tail_hooks and n in tail_hooks:
                    for fn in tail_hooks[n]:
                        fn()
            return yt

        def dump(tiles, width=S):
            # debug: write four [128, >=width] tiles to yt_out (via staging)
            for i, t in enumerate(tiles[:4]):
                yo = pool.tile([128, S], bf16, tag="out", bufs=2, name=f"dmp{i}")
                nc.vector.tensor_copy(yo[:, 0:width], t[:, 0:width])
                nc.sync.dma_start(yt_out[i * 128:(i + 1) * 128, 0:width],
                                  yo[:, 0:width])

        if upto == 0:                       # stage A: x0 passthrough
            xt = [pool.tile([128, S], bf16, tag="featmaj", bufs=16,
                            name=f"a{k}") for k in range(KH)]
            for k in range(KH):
                for s in range(SC):
                    nc.vector.tensor_copy(xt[k][:, s * 128:(s + 1) * 128],
                                          x0sb[s][:, k * 128:(k + 1) * 128])
            dump(xt)
        if upto >= 1:
            y1 = dense1()
            if upto == 1:
                dump(y1)
        if upto >= 2:
            mix2_panels = []
            w2pans = []
            hooks1 = {
                1: [lambda: w2pans.append(w2_panel(0))],
                3: [lambda: w2pans.append(w2_panel(1))],
            }
            y1sq = mix(y1, m1, m1pan, B1, "y1sq", tail_hooks=hooks1)
            if upto == 2:
                dump(y1sq)
        if upto >= 3:
            w3sb = pool.tile([128, KI * 512], bf16)
            hooks2 = {
                0: [lambda: mix2_panels.append(m_panel(m2, 0, "m2"))],
                1: [lambda: mix2_panels.append(m_panel(m2, 1, "m2"))],
                2: [lambda: nc.sync.dma_start(w3sb[:], w3[:]),
                    lambda: mix2_panels.append(m_panel(m2, 2, "m2"))],
            }
            y2 = dense2(y1sq, w2pans, tail_hooks=hooks2)
            if upto == 3:
                dump(y2)
        if upto >= 4:
            y2sq = mix(y2, m2, mix2_panels, B2, "y2sq")
            if upto == 4:
                dump(y2sq)
        if upto >= 5:                       # dense3 + bias + store
            for ht in range(HT):
                yo = pool.tile([128, S], bf16, tag="out", bufs=2, name=f"yt{ht}")
                # run the two half-tiles sequentially so the first half's
                # drain + store hides under the second half's matmuls; only
                # the last half's drain remains on the kernel tail
                pss = [psum.tile([128, 512], f32, tag="mm", bufs=8,
                                 name=f"p3_{ht}_{sc}") for sc in range(2)]
                for sc in range(2):
                    for k in range(KI):
                        nc.tensor.matmul(
                            pss[sc][:],
                            w3sb[:, k * 512 + ht * 128: k * 512 + ht * 128 + 128],
                            y2sq[k][:, sc * 512:(sc + 1) * 512],
                            start=(k == 0), stop=(k == KI - 1))
                    if sc == 0:
                        nc.vector.tensor_scalar_add(yo[:, 0:512], pss[0][:],
                                                    bc(B3, ht))
                    elif ht == HT - 1:
                        # very last drain: split across DVE+ACT so the two
                        # quarter-drains run in parallel on the kernel tail
                        nc.vector.tensor_scalar_add(yo[:, 512:768],
                                                    pss[1][:, 0:256], bc(B3, ht))
                        nc.scalar.activation(yo[:, 768:1024], pss[1][:, 256:512],
                                             AF.Identity, bias=bc(B3, ht))
                    else:
                        nc.scalar.activation(yo[:, 512:1024], pss[1][:],
                                             AF.Identity, bias=bc(B3, ht))
                    nc.sync.dma_start(
                        yt_out[ht * 128:(ht + 1) * 128, sc * 512:(sc + 1) * 512],
                        yo[:, sc * 512:(sc + 1) * 512])

    nc.compile()
    return nc


_PROGRAMS = {}
LAST_RESULT = None


def _get_program(stage="full"):
    if stage not in _PROGRAMS:
        _PROGRAMS[stage] = _build_program(stage)
    return _PROGRAMS[stage]


def _prep_maps(x, word_emb, pos_emb, tok_emb, emb_ln_w, emb_ln_b,
               init_d, init_b, init_M, inter0_d, inter0_b, inter0_M,
               final_d, final_b):
    x = np.asarray(x)
    f = lambda a: np.ascontiguousarray(np.asarray(a), dtype=np.float32)
    h = lambda a: np.ascontiguousarray(a).astype(ml_dtypes.bfloat16)
    # embedding + layernorm folded into input packing (host side)
    emb = f(word_emb)[x] + f(pos_emb)[None] + f(tok_emb)[0][None, None]
    mu = emb.mean(-1, keepdims=True)
    var = ((emb - mu) ** 2).mean(-1, keepdims=True)
    x0 = (emb - mu) / np.sqrt(var + EPS) * f(emb_ln_w) + f(emb_ln_b)  # (B,S,HID)
    x0t = x0.transpose(0, 2, 1)                                       # (B,HID,S)
    x0sh = x0t.reshape(B, KH, 128, SC, 128).transpose(0, 2, 3, 1, 4) \
        .reshape(B, 128, SC * KH * 128)
    w1h = f(init_d).reshape(NC1, 512, KH, 128).transpose(0, 3, 2, 1) \
        .reshape(NC1, 128, KH * 512).transpose(1, 0, 2).reshape(128, NC1 * KH * 512)
    w2h = f(inter0_d).reshape(NC1, 512, KI, 128).transpose(0, 3, 2, 1) \
        .reshape(NC1 * 128, KI * 512)
    w3h = f(final_d).reshape(HID, KI, 128).transpose(2, 1, 0) \
        .reshape(128, KI * 512)
    m1h = f(init_M).reshape(NH, SC, 128, S).transpose(0, 2, 1, 3) \
        .reshape(NH * 128, SC * S)
    m2h = f(inter0_M).reshape(NH, SC, 128, S).transpose(0, 2, 1, 3) \
        .reshape(NH * 128, SC * S)
    bcat = np.concatenate([
        f(init_b).reshape(KI, 128).T,
        f(inter0_b).reshape(KI, 128).T,
        f(final_b).reshape(HT, 128).T,
    ], axis=1)
    shared = dict(
        bcat=np.ascontiguousarray(bcat),
        w1=h(w1h), w2=h(w2h), w3=h(w3h), m1=h(m1h), m2=h(m2h),
    )
    in_maps = []
    for b in range(B):
        in_maps.append(dict(shared, x0s=h(x0sh[b])))
    return in_maps


def kernel(**inputs):
    global LAST_RESULT
    stage = os.environ.get("KSTAGE", "full")
    ncores = int(os.environ.get("KCORES", str(N_CORES)))
    in_maps = _prep_maps(**inputs)
    in_maps = in_maps[:ncores]
    nc = _get_program(stage)
    res = run_bass_kernel_spmd(nc, in_maps, list(range(ncores)))
    LAST_RESULT = res
    out = np.stack([np.asarray(res.results[b]["yt"], dtype=np.float32).T
                    for b in range(ncores)])
    if ncores < B:
        out = np.concatenate([out] + [out[:1]] * (B - ncores))
    return out


# revision 24
# speedup vs baseline: 1.0022x; 1.0022x over previous
"""BertTinyFlatten on 8 Trainium2 NeuronCores — data-parallel over batch.

Host prep folds the (tiny, ~0.01% of FLOPs) embedding gather + layernorm
into input packing: each core receives x0t = LN(word_emb[x]+pos+tok)^T
pre-transposed into feature-major s-chunk panels, plus the five matmul
operands host-prelaid in PE-friendly layouts.  The device program is a
pure matmul pipeline (99.99% of the reference FLOPs):

  y1    = x0 @ init_d.T            (token-major out)      [bf16 matmuls]
  y1sq  = (mix(y1, init_M) + b1)^2   -> feature-major     [ACT Square fused]
  y2    = y1sq @ inter0_d.T          -> token-major
  y2sq  = (mix(y2, inter0_M) + b2)^2 -> feature-major
  yt    = final_d @ y2sq + b3        -> feature-major; host transposes back

All matmul operands are bf16 (PSUM accumulates fp32): the moving operand
streams 1 col/cycle and the 128-col LDWEIGHTS rides the background weight
buffer, giving the 216 ns warm pitch at N=512.  PSUM->SBUF drains
alternate between the Scalar and Vector engines so neither queue backs up
ahead of the next stage's gated matmuls.  All DMA triggers ride the Sync
engine stream: triggers issue in program order and the HWDGE queue is
FIFO, which is what keeps the bulk panels (m1/m2 ring-prefetched 3 deep,
w2/w3 from mix1/dense2 tail hooks) behind the critical dense1 operands —
a dep-free trigger on any other engine gets hoisted to t=0 by the Tile
scheduler and starves the critical path.  A short burst of junk matmuls
ramps the PE clock (DVFS) while the first operands land, so dense1 runs
at the warm pitch from its first matmul.  Mid-kernel the PE has zero
gaps; the remaining overhead is first-operand DMA latency (~6.5us) and
the fixed runtime teardown (~11us).
"""
import os
import sys

import numpy as np
import ml_dtypes

for _p in ("/opt/trn_rl_repo", "/opt/pypackages"):
    if _p not in sys.path and os.path.isdir(_p):
        sys.path.append(_p)

from contextlib import ExitStack

import concourse.bass as bass
import concourse.tile as tile
from concourse import bacc, masks, mybir
from concourse.bass_utils import run_bass_kernel_spmd

f32 = mybir.dt.float32
bf16 = mybir.dt.bfloat16
AF = mybir.ActivationFunctionType
ALU = mybir.AluOpType

B, S, HID, NH, INTER, VOCAB = 8, 1024, 512, 8, 2048, 30522
DH = INTER // NH            # 256 features per head
EPS = 1e-12
N_CORES = 8

KH = HID // 128             # 4   k-tiles for dense1
KI = INTER // 128           # 16  k-tiles for dense2/3
SC = S // 128               # 8   token chunks
NC1 = INTER // 512          # 4   n-chunks (512) for dense1/2
HT = HID // 128             # 4   hid tiles for dense3

STAGES = ("A", "B", "C", "D", "E", "full")


def _build_program(stage="full"):
    upto = STAGES.index(stage)
    nc = bacc.Bacc("TRN2", target_bir_lowering=False, debug=False,
                   num_devices=N_CORES)

    # prelaid panels (host does the tiling): see _prep_maps for layouts
    x0s = nc.dram_tensor("x0s", [128, SC * KH * 128], bf16,
                         kind="ExternalInput").ap()
    # biases catted: [0:16]=b1, [16:32]=b2, [32:36]=b3
    bcat = nc.dram_tensor("bcat", [128, 2 * KI + HT], f32,
                          kind="ExternalInput").ap()
    w1 = nc.dram_tensor("w1", [128, NC1 * KH * 512], bf16, kind="ExternalInput").ap()
    m1 = nc.dram_tensor("m1", [NH * 128, SC * S], bf16, kind="ExternalInput").ap()
    w2 = nc.dram_tensor("w2", [NC1 * 128, KI * 512], bf16, kind="ExternalInput").ap()
    m2 = nc.dram_tensor("m2", [NH * 128, SC * S], bf16, kind="ExternalInput").ap()
    w3 = nc.dram_tensor("w3", [128, KI * 512], bf16, kind="ExternalInput").ap()
    yt_out = nc.dram_tensor("yt", [HID, S], bf16, kind="ExternalOutput").ap()

    with tile.TileContext(nc) as tc, ExitStack() as ctx:
        pool = ctx.enter_context(tc.tile_pool(name="sbuf", bufs=1))
        psum = ctx.enter_context(tc.tile_pool(name="psum", bufs=1, space="PSUM"))

        # ---- critical-path DMAs first (Sync HWDGE queue, FIFO), ordered
        # by first consumption.  dense1 runs n-outer/s-inner, so pass 0
        # needs x0s chunks progressively (128KB each) and each later w1
        # n-panel has a full 6.9us pass of runway.  All triggers ride the
        # Sync engine stream: it issues in program order and the queue is
        # FIFO, which keeps the m panels behind the dense1 operands (the
        # Tile scheduler hoists dep-free triggers on any other engine to
        # t=0, starving the critical path).  Biases are first read ~40us in.
        x0sb = [pool.tile([128, KH * 128], bf16, tag="x0s", bufs=SC,
                          name=f"x0s{s}") for s in range(SC)]
        w1sb = pool.tile([128, NC1 * KH * 512], bf16)
        nc.sync.dma_start(x0sb[0][:], x0s[:, 0:512])
        nc.sync.dma_start(w1sb[:, 0:2048], w1[:, 0:2048])
        for s in range(1, SC):
            nc.sync.dma_start(x0sb[s][:], x0s[:, s * 512:(s + 1) * 512])
        for n in range(1, NC1):
            nc.sync.dma_start(w1sb[:, n * 2048:(n + 1) * 2048],
                              w1[:, n * 2048:(n + 1) * 2048])

        # m panels ride the same FIFO queue behind the dense1 operands; the
        # bufs=3 ring WAR-gates the h+3 refill triggers during the mixes.
        def m_panel(m_ap, h, nm):
            t = pool.tile([128, SC * S], bf16, tag="m", bufs=3, name=f"{nm}p{h}")
            nc.sync.dma_start(t[:], m_ap[h * 128:(h + 1) * 128, :])
            return t

        m1pan = [m_panel(m1, 0, "m1")]
        t_bc = pool.tile([128, 2 * KI + HT], f32)
        nc.sync.dma_start(t_bc[:], bcat[:])
        m1pan += [m_panel(m1, 1, "m1"), m_panel(m1, 2, "m1")]

        # ---- constants + PE/ACT warmup ----------------------------------
        ident = pool.tile([128, 128], bf16)
        masks.make_identity(nc, ident[:])
        zerocol = pool.tile([128, 1], f32)
        nc.vector.memset(zerocol[:], 0.0)
        junk = pool.tile([128, 512], bf16)
        nc.vector.memset(junk[:], 0.0)
        # column bases into t_bc: b1, b2, b3
        B1, B2, B3 = 0, KI, 2 * KI

        def bc(base, i):
            return t_bc[:, base + i:base + i + 1]

        # dummy Square so walrus loads the square/identity table set once,
        # off the critical path
        actwarm = pool.tile([128, 1], f32)
        nc.scalar.activation(actwarm[:], zerocol[:], AF.Square, bias=zerocol[:])

        # junk matmuls ramp the PE clock while x0s0/w1n0 land (~4.5us at
        # the slow early DMA rate); sized to cover the arrival jitter so the
        # PE never idles before dense1 — an idle gap there lets the DVFS
        # clock sag and costs ~2.5us of half-speed dense1 matmuls
        warmps = psum.tile([128, 512], f32, tag="mm", bufs=8, name="warm")
        for i in range(11):
            nc.tensor.matmul(warmps[:], ident[:], junk[:], start=True, stop=True)

        # feature-major activations live in the 16-slot "featmaj" ring:
        # y1sq (16) -> y2sq (16), WAR-serialized by Tile.
        def mix(yin, m_ap, panels, bias_base, out_name, tail_hooks=None):
            # per-head seq mix + bias + square; token-major in, feature-major out
            ysq = []
            for h in range(NH):
                pan = panels[h]
                groups = [[None] * 2 for _ in range(2)]
                for tc_i in range(2):
                    for dp in range(2):
                        groups[tc_i][dp] = psum.tile(
                            [128, 512], f32, tag="mm", bufs=8,
                            name=f"{out_name}p{h}_{tc_i}_{dp}")
                for s in range(SC):
                    for dp in range(2):
                        lhsT = yin[s][:, h * DH + dp * 128: h * DH + (dp + 1) * 128]
                        for tc_i in range(2):
                            nc.tensor.matmul(
                                groups[tc_i][dp][:], lhsT,
                                pan[:, s * S + tc_i * 512: s * S + (tc_i + 1) * 512],
                                start=(s == 0), stop=(s == SC - 1))
                if h + 3 < NH:
                    panels.append(m_panel(m_ap, h + 3, out_name))
                if tail_hooks and h in tail_hooks:
                    for fn in tail_hooks[h]:
                        fn()
                for dp in range(2):
                    i = h * 2 + dp
                    yo = pool.tile([128, S], bf16, tag="featmaj", bufs=16,
                                   name=f"{out_name}{i}")
                    for tc_i in range(2):
                        nc.scalar.activation(yo[:, tc_i * 512:(tc_i + 1) * 512],
                                             groups[tc_i][dp][:], AF.Square,
                                             bias=bc(bias_base, i))
                    ysq.append(yo)
            return ysq

        def w2_panel(n):
            # all bulk DMAs ride the Sync engine stream: the engine issues
            # triggers in program order and the HWDGE queue is FIFO, which is
            # the only thing keeping the panel stream from starving the
            # critical dense1 operands (the Tile scheduler hoists dep-free
            # triggers on any other engine to t=0)
            t = pool.tile([128, KI * 512], bf16, tag="w2", bufs=2, name=f"w2q{n}")
            nc.sync.dma_start(t[:], w2[n * 128:(n + 1) * 128, :])
            return t

        def dense1():
            # token-major out: y1[s] = x0[s-chunk] @ w1; n-outer so each w1
            # n-panel DMA has a full pass of matmul runway
            yt = []
            for s in range(SC):
                yt.append(pool.tile([128, INTER], bf16, tag="tokmaj", bufs=SC,
                                    name=f"y1_{s}"))
            for n in range(NC1):
                for s in range(SC):
                    ps = psum.tile([128, 512], f32, tag="mm", bufs=8,
                                   name=f"y1p{n}_{s}")
                    for k in range(KH):
                        nc.tensor.matmul(
                            ps[:], x0sb[s][:, k * 128:(k + 1) * 128],
                            w1sb[:, n * 2048 + k * 512: n * 2048 + (k + 1) * 512],
                            start=(k == 0), stop=(k == KH - 1))
                    if s % 2:
                        nc.scalar.copy(yt[s][:, n * 512:(n + 1) * 512], ps[:])
                    else:
                        nc.vector.tensor_copy(yt[s][:, n * 512:(n + 1) * 512],
                                              ps[:])
            return yt

        def dense2(xin, w2pans, tail_hooks=None):
            yt = []
            for s in range(SC):
                yt.append(pool.tile([128, INTER], bf16, tag="tokmaj", bufs=SC,
                                    name=f"y2_{s}"))
            for n in range(NC1):
                wq = w2pans[n]
                for s in range(SC):
                    ps = psum.tile([128, 512], f32, tag="mm", bufs=8,
                                   name=f"y2p{n}_{s}")
                    for k in range(KI):
                        nc.tensor.matmul(
                            ps[:], xin[k][:, s * 128:(s + 1) * 128],
                            wq[:, k * 512:(k + 1) * 512],
                            start=(k == 0), stop=(k == KI - 1))
                    if s % 2:
                        nc.scalar.copy(yt[s][:, n * 512:(n + 1) * 512], ps[:])
                    else:
                        nc.vector.tensor_copy(yt[s][:, n * 512:(n + 1) * 512],
                                              ps[:])
                if n + 2 < NC1:
                    w2pans.append(w2_panel(n + 2))
                if tail_hooks and n in tail_hooks:
                    for fn in tail_hooks[n]:
                        fn()
            return yt

        def dump(tiles, width=S):
            # debug: write four [128, >=width] tiles to yt_out (via staging)
            for i, t in enumerate(tiles[:4]):
                yo = pool.tile([128, S], bf16, tag="out", bufs=2, name=f"dmp{i}")
                nc.vector.tensor_copy(yo[:, 0:width], t[:, 0:width])
                nc.sync.dma_start(yt_out[i * 128:(i + 1) * 128, 0:width],
                                  yo[:, 0:width])

        if upto == 0:                       # stage A: x0 passthrough
            xt = [pool.tile([128, S], bf16, tag="featmaj", bufs=16,
                            name=f"a{k}") for k in range(KH)]
            for k in range(KH):
                for s in range(SC):
                    nc.vector.tensor_copy(xt[k][:, s * 128:(s + 1) * 128],
                                          x0sb[s][:, k * 128:(k + 1) * 128])
            dump(xt)
        if upto >= 1:
            y1 = dense1()
            if upto == 1:
                dump(y1)
        if upto >= 2:
            mix2_panels = []
            w2pans = []
            hooks1 = {
                1: [lambda: w2pans.append(w2_panel(0))],
                3: [lambda: w2pans.append(w2_panel(1))],
            }
            y1sq = mix(y1, m1, m1pan, B1, "y1sq", tail_hooks=hooks1)
            if upto == 2:
                dump(y1sq)
        if upto >= 3:
            w3sb = pool.tile([128, KI * 512], bf16)
            hooks2 = {
                0: [lambda: mix2_panels.append(m_panel(m2, 0, "m2"))],
                1: [lambda: mix2_panels.append(m_panel(m2, 1, "m2"))],
                2: [lambda: nc.sync.dma_start(w3sb[:], w3[:]),
                    lambda: mix2_panels.append(m_panel(m2, 2, "m2"))],
            }
            y2 = dense2(y1sq, w2pans, tail_hooks=hooks2)
            if upto == 3:
                dump(y2)
        if upto >= 4:
            y2sq = mix(y2, m2, mix2_panels, B2, "y2sq")
            if upto == 4:
                dump(y2sq)
        if upto >= 5:                       # dense3 + bias + store
            for ht in range(HT):
                yo = pool.tile([128, S], bf16, tag="out", bufs=2, name=f"yt{ht}")
                # run the two half-tiles sequentially so the first half's
                # drain + store hides under the second half's matmuls; only
                # the last half's drain remains on the kernel tail
                pss = [psum.tile([128, 512], f32, tag="mm", bufs=8,
                                 name=f"p3_{ht}_{sc}") for sc in range(2)]
                for sc in range(2):
                    for k in range(KI):
                        nc.tensor.matmul(
                            pss[sc][:],
                            w3sb[:, k * 512 + ht * 128: k * 512 + ht * 128 + 128],
                            y2sq[k][:, sc * 512:(sc + 1) * 512],
                            start=(k == 0), stop=(k == KI - 1))
                    if sc == 0:
                        nc.vector.tensor_scalar_add(yo[:, 0:512], pss[0][:],
                                                    bc(B3, ht))
                    elif ht == HT - 1:
                        # very last drain goes to DVE: its queue is empty at
                        # kernel end, while the Scalar queue has end-of-kernel
                        # semaphore checks scheduled ahead of a final ACTIVATE
                        # (observed ~0.6us head-of-line delay)
                        nc.vector.tensor_scalar_add(yo[:, 512:1024], pss[1][:],
                                                    bc(B3, ht))
                    else:
                        nc.scalar.activation(yo[:, 512:1024], pss[1][:],
                                             AF.Identity, bias=bc(B3, ht))
                    nc.sync.dma_start(
                        yt_out[ht * 128:(ht + 1) * 128, sc * 512:(sc + 1) * 512],
                        yo[:, sc * 512:(sc + 1) * 512])

    nc.compile()
    return nc


_PROGRAMS = {}
LAST_RESULT = None


def _get_program(stage="full"):
    if stage not in _PROGRAMS:
        _PROGRAMS[stage] = _build_program(stage)
    return _PROGRAMS[stage]


def _prep_maps(x, word_emb, pos_emb, tok_emb, emb_ln_w, emb_ln_b,
               init_d, init_b, init_M, inter0_d, inter0_b, inter0_M,
               final_d, final_b):
    x = np.asarray(x)
    f = lambda a: np.ascontiguousarray(np.asarray(a), dtype=np.float32)
    h = lambda a: np.ascontiguousarray(a).astype(ml_dtypes.bfloat16)
    # embedding + layernorm folded into input packing (host side)
    emb = f(word_emb)[x] + f(pos_emb)[None] + f(tok_emb)[0][None, None]
    mu = emb.mean(-1, keepdims=True)
    var = ((emb - mu) ** 2).mean(-1, keepdims=True)
    x0 = (emb - mu) / np.sqrt(var + EPS) * f(emb_ln_w) + f(emb_ln_b)  # (B,S,HID)
    x0t = x0.transpose(0, 2, 1)                                       # (B,HID,S)
    x0sh = x0t.reshape(B, KH, 128, SC, 128).transpose(0, 2, 3, 1, 4) \
        .reshape(B, 128, SC * KH * 128)
    w1h = f(init_d).reshape(NC1, 512, KH, 128).transpose(0, 3, 2, 1) \
        .reshape(NC1, 128, KH * 512).transpose(1, 0, 2).reshape(128, NC1 * KH * 512)
    w2h = f(inter0_d).reshape(NC1, 512, KI, 128).transpose(0, 3, 2, 1) \
        .reshape(NC1 * 128, KI * 512)
    w3h = f(final_d).reshape(HID, KI, 128).transpose(2, 1, 0) \
        .reshape(128, KI * 512)
    m1h = f(init_M).reshape(NH, SC, 128, S).transpose(0, 2, 1, 3) \
        .reshape(NH * 128, SC * S)
    m2h = f(inter0_M).reshape(NH, SC, 128, S).transpose(0, 2, 1, 3) \
        .reshape(NH * 128, SC * S)
    bcat = np.concatenate([
        f(init_b).reshape(KI, 128).T,
        f(inter0_b).reshape(KI, 128).T,
        f(final_b).reshape(HT, 128).T,
    ], axis=1)
    shared = dict(
        bcat=np.ascontiguousarray(bcat),
        w1=h(w1h), w2=h(w2h), w3=h(w3h), m1=h(m1h), m2=h(m2h),
    )
    in_maps = []
    for b in range(B):
        in_maps.append(dict(shared, x0s=h(x0sh[b])))
    return in_maps


def kernel(**inputs):
    global LAST_RESULT
    stage = os.environ.get("KSTAGE", "full")
    ncores = int(os.environ.get("KCORES", str(N_CORES)))
    in_maps = _prep_maps(**inputs)
    in_maps = in_maps[:ncores]
    nc = _get_program(stage)
    res = run_bass_kernel_spmd(nc, in_maps, list(range(ncores)))
    LAST_RESULT = res
    out = np.stack([np.asarray(res.results[b]["yt"], dtype=np.float32).T
                    for b in range(ncores)])
    if ncores < B:
        out = np.concatenate([out] + [out[:1]] * (B - ncores))
    return out
